# revision 32
# baseline (speedup 1.0000x reference)
"""Trainium2 Bass kernel for nn_MixtureOfExperts (B=E=H=IN=256, 4 heads).

Strategy (8 NeuronCores, expert-parallel):
  - Each core owns E_loc=32 experts' W1/W2 (host pre-transposed so the
    contraction dim lands on SBUF partitions; streamed as moving operands).
  - Per expert on-device: h = x@W1e^T (+b1) -> LayerNorm (bn_stats/bn_aggr,
    fused normalize+ReLU on ACT) -> PE-transpose -> eo^T = W2e^T-stationary
    matmul (+b2 via ACT bias) -> k~ = Wk@eo^T -> scores via a head-mask
    matmul (folds the q.k head dot-products into one PE op).
  - Scores AllGather (128KB) -> replicated softmax/head-mean -> weights.
  - Combine: per-expert fused multiply-accumulate (scalar_tensor_tensor) of
    eo^T slices against weight columns selected by a per-core one-hot `sel`
    matrix (avoids rank-dependent addressing) -> AllReduce -> final
    Linear+LayerNorm+ReLU replicated on every core.
  - Matmuls run in float32r (4-byte storage, ~4x faster PE path, ~1e-4 rel
    rounding); operands are cast to f32r by SWDGE DMA or written as f32r by
    ACT/DVE. Scores transpose stays exact fp32.
"""

import sys

sys.path.insert(0, "/opt/trn_rl_repo")

import numpy as np
from concourse import mybir
import concourse.bass as bass
import concourse.bacc as bacc
import concourse.tile as tile
from concourse.bass_utils import run_bass_kernel_spmd

P = 128
B = H = IN = 256
NH = 4
HD = 64
NCORES = 8
ELOC = 32
EPS = 1e-5
F32 = mybir.dt.float32
F32R = mybir.dt.float32r
ts = bass.ts
AF = mybir.ActivationFunctionType
ALU = mybir.AluOpType


def build_program(use_b1, use_b2, use_affine, use_cb, use_f32r=True):
    DT = F32R if use_f32r else F32
    nc = bacc.Bacc("TRN2", target_bir_lowering=False, num_devices=NCORES)

    def ld(out, in_):
        # f32r operands are pre-cast on the host; plain HWDGE raw loads
        return nc.sync.dma_start(out=out, in_=in_)

    def inp(name, shape, dt=F32):
        return nc.dram_tensor(name, shape, dt, kind="ExternalInput")

    DTI = F32R if use_f32r else F32
    xT = inp("xT", [IN, B], DTI)
    teT = inp("teT", [IN, B], DTI)
    # [pair, i, j*o]: pre-interleaved expert pairs (j in {0,1} selects expert)
    w1t = inp("w1t", [ELOC // 2, IN, 2 * H], DTI)
    w2t = inp("w2t", [ELOC // 2, H, 2 * H], DTI)
    wqT = inp("wqT", [IN, H], DTI)
    wkT = inp("wkT", [H, H], DTI)
    cwT = inp("cwT", [H, H], DTI)
    bqs = inp("bqs", [2, P, 1])      # (bq/8) as column tiles
    bks = inp("bks", [2, P, 1])
    b2t = inp("b2t", [2, P, ELOC])
    Lm = inp("Lm", [2, P, NH], DTI)
    sel = inp("sel", [B, ELOC], DTI)
    ident = inp("ident", [P, P])
    identr = inp("identr", [P, P], DTI)
    ones_row = inp("ones_row", [1, P], DTI)
    cg_row = inp("cg_row", [1, H])
    cbeta_row = inp("cbeta_row", [1, H])
    if use_b1:
        b1r = inp("b1r", [ELOC // 2, 1, 2 * H], DTI)
    if use_affine:
        lng = inp("lng", [ELOC, H])
        lnb = inp("lnb", [ELOC, H])
    if use_cb:
        cb_row = inp("cb_row", [1, H], DTI)

    eo_out = nc.dram_tensor("eo_out", [ELOC, H, B], F32, kind="ExternalOutput")
    w_out = nc.dram_tensor("w_out", [B, B], F32, kind="ExternalOutput")
    y_out = nc.dram_tensor("y_out", [B // NCORES, H], F32, kind="ExternalOutput")

    with tile.TileContext(nc) as tc:
        with (
            tc.tile_pool(name="singles", bufs=1) as singles,
            tc.tile_pool(name="wts", bufs=3) as wts,
            tc.tile_pool(name="work", bufs=3) as work,
            tc.tile_pool(name="stats", bufs=4) as stats,
            tc.tile_pool(name="ps_h", bufs=3, space="PSUM") as ps_h,
            tc.tile_pool(name="ps_big", bufs=4, space="PSUM") as ps_big,
            tc.tile_pool(name="ps_sc", bufs=1, space="PSUM") as ps_sc,
            tc.tile_pool(name="dram", bufs=1, space="DRAM") as dram,
        ):
            # ---- static loads (DT tiles feed matmuls) ----
            def load2(name, t, cols):
                s = singles.tile([P, 2, cols], DT, tag=name)
                ld(s[:], t.rearrange("(k p) c -> p k c", p=P))
                return s

            xt_sb = load2("xt", xT, B)
            tet_sb = load2("tet", teT, B)
            wqt_sb = load2("wqt", wqT, H)
            wkt_sb = load2("wkt", wkT, H)
            cwt_sb = load2("cwt", cwT, H)
            sel_sb = load2("sel", sel, ELOC)

            bq_sb = singles.tile([P, 2, 1], F32, tag="bq")
            nc.sync.dma_start(out=bq_sb[:], in_=bqs.rearrange("k p c -> p k c"))
            bk_sb = singles.tile([P, 2, 1], F32, tag="bk")
            nc.sync.dma_start(out=bk_sb[:], in_=bks.rearrange("k p c -> p k c"))
            b2_sb = singles.tile([P, 2, ELOC], F32, tag="b2")
            nc.sync.dma_start(out=b2_sb[:], in_=b2t.rearrange("k p c -> p k c"))
            Lm_sb = singles.tile([P, 2, NH], DT, tag="Lm")
            ld(Lm_sb[:], Lm.rearrange("k p c -> p k c"))
            ident_r = singles.tile([P, P], DT, tag="ident_r")
            ld(ident_r[:], identr[:])
            ident_f = singles.tile([P, P], F32, tag="ident_f")
            nc.sync.dma_start(out=ident_f[:], in_=ident[:])
            ones1 = singles.tile([1, P], DT, tag="ones1")
            ld(ones1[:], ones_row[:])

            eps_sb = singles.tile([P, 1], F32, tag="eps")
            nc.vector.memset(eps_sb[:], EPS)

            def bcast_row(t_ap, name):
                # [1, H] DRAM row -> [P, H] SBUF (replicated across partitions)
                s = singles.tile([P, H], F32, tag=name)
                src = bass.AP(
                    tensor=t_ap.tensor,
                    offset=t_ap.offset,
                    ap=[[0, P]] + list(t_ap.ap[1:]),
                )
                nc.gpsimd.dma_start(out=s[:], in_=src)
                return s

            cg_bc = bcast_row(cg_row[:], "cg_bc")
            cbeta_bc = bcast_row(cbeta_row[:], "cbeta_bc")
            if use_cb:
                cb_sb = singles.tile([1, H], DT, tag="cb")
                ld(cb_sb[:], cb_row[:])

            eoT_sb = singles.tile([P, 2, ELOC, B], DT, tag="eoT")
            qt_sb = singles.tile([P, 2, B], F32, tag="qt")
            sc_stage = singles.tile([NH, ELOC, B], F32, tag="sc_stage")

            # ---- q projection: qT = (Wq @ te^T)*0.125 + bq*0.125 ----
            for oi in range(2):
                qp = ps_h.tile([P, B], F32, tag="h")
                for ki in range(2):
                    nc.tensor.matmul(
                        qp[:],
                        lhsT=wqt_sb[:, ki, ts(oi, P)],
                        rhs=tet_sb[:, ki, :],
                        start=(ki == 0),
                        stop=(ki == 1),
                    )
                nc.scalar.activation(
                    out=qt_sb[:, oi, :],
                    in_=qp[:],
                    func=AF.Identity,
                    scale=0.125,
                    bias=bq_sb[:, oi, :],
                )

            # ---- expert loop (pairs: N=512 matmuls, full-bank PSUM) ----
            for ep in range(ELOC // 2):
                e0 = 2 * ep
                w1_t = wts.tile([P, 2, 2 * H], DT, tag="w1")
                ld(w1_t[:], w1t[ep].rearrange("(k p) c -> p k c", p=P))
                w2_t = wts.tile([P, 2, 2 * H], DT, tag="w2")
                nc.gpsimd.dma_start(
                    out=w2_t[:], in_=w2t[ep].rearrange("(k p) c -> p k c", p=P)
                )
                a_sb = work.tile([P, 2, 2 * H], DT, tag="a")
                if use_b1:
                    b1_t = work.tile([1, 2 * H], DT, tag="b1t")
                    ld(b1_t[:], b1r[ep])
                h_pss = []
                mv2 = stats.tile([P, 2, 2, 2], F32, tag="mv")
                for mi in range(2):
                    h_ps = ps_h.tile([P, 2 * H], F32, tag="h")
                    h_pss.append(h_ps)
                    for ki in range(2):
                        nc.tensor.matmul(
                            h_ps[:],
                            lhsT=xt_sb[:, ki, ts(mi, P)],
                            rhs=w1_t[:, ki, :],
                            start=(ki == 0),
                            stop=(ki == 1) and not use_b1,
                        )
                    if use_b1:
                        nc.tensor.matmul(
                            h_ps[:], lhsT=ones1[:], rhs=b1_t[:],
                            start=False, stop=True,
                        )
                    for el in range(2):
                        st = stats.tile([P, 6], F32, tag="st")
                        nc.vector.bn_stats(out=st[:], in_=h_ps[:, ts(el, H)])
                        nc.vector.bn_aggr(out=mv2[:, mi, el, :], in_=st[:])
                # batched normalize math: rstd/-mu*rstd for all (mi, el)
                rstd2 = stats.tile([P, 2, 2], F32, tag="rstd")
                nc.scalar.activation(
                    out=rstd2[:].rearrange("p a b -> p (a b)"),
                    in_=mv2[:].rearrange("p a b c -> p (a b) c")[:, :, 1],
                    func=AF.Sqrt,
                    bias=eps_sb[:],
                )
                nc.vector.reciprocal(
                    out=rstd2[:].rearrange("p a b -> p (a b)"),
                    in_=rstd2[:].rearrange("p a b -> p (a b)"),
                )
                nmr2 = stats.tile([P, 2, 2], F32, tag="nmr")
                nc.vector.scalar_tensor_tensor(
                    out=nmr2[:].rearrange("p a b -> p (a b)"),
                    in0=mv2[:].rearrange("p a b c -> p (a b) c")[:, :, 0],
                    scalar=-1.0,
                    in1=rstd2[:].rearrange("p a b -> p (a b)"),
                    op0=ALU.mult,
                    op1=ALU.mult,
                )
                for mi in range(2):
                    for el in range(2):
                        if not use_affine:
                            nc.scalar.activation(
                                out=a_sb[:, mi, ts(el, H)],
                                in_=h_pss[mi][:, ts(el, H)],
                                func=AF.Relu,
                                scale=rstd2[:, mi, el : el + 1],
                                bias=nmr2[:, mi, el : el + 1],
                            )
                        else:
                            nc.scalar.activation(
                                out=a_sb[:, mi, ts(el, H)],
                                in_=h_pss[mi][:, ts(el, H)],
                                func=AF.Identity,
                                scale=rstd2[:, mi, el : el + 1],
                                bias=nmr2[:, mi, el : el + 1],
                            )
                            e = e0 + el
                            g_bc = work.tile([P, H], F32, tag="g_bc")
                            src = lng[e : e + 1, :]
                            nc.gpsimd.dma_start(
                                out=g_bc[:],
                                in_=bass.AP(
                                    tensor=src.tensor, offset=src.offset,
                                    ap=[[0, P]] + list(src.ap[1:]),
                                ),
                            )
                            b_bc = work.tile([P, H], F32, tag="b_bc")
                            src = lnb[e : e + 1, :]
                            nc.gpsimd.dma_start(
                                out=b_bc[:],
                                in_=bass.AP(
                                    tensor=src.tensor, offset=src.offset,
                                    ap=[[0, P]] + list(src.ap[1:]),
                                ),
                            )
                            nc.vector.tensor_mul(
                                a_sb[:, mi, ts(el, H)], a_sb[:, mi, ts(el, H)], g_bc[:]
                            )
                            nc.vector.tensor_add(
                                a_sb[:, mi, ts(el, H)], a_sb[:, mi, ts(el, H)], b_bc[:]
                            )
                            nc.vector.tensor_scalar_max(
                                a_sb[:, mi, ts(el, H)], a_sb[:, mi, ts(el, H)], 0.0
                            )
                # transpose a -> aT [h, (el, b)]
                aT_sb = work.tile([P, 2, 2, B], DT, tag="aT")
                for ki in range(2):
                    aT_ps = ps_big.tile([P, 2 * B], DT, tag="big")
                    for el in range(2):
                        for mi in range(2):
                            nc.tensor.transpose(
                                aT_ps[:, el * B + mi * P : el * B + (mi + 1) * P],
                                a_sb[:, mi, el * H + ki * P : el * H + (ki + 1) * P],
                                ident_r[:],
                            )
                    if ki == 0:
                        nc.vector.tensor_copy(aT_sb[:, ki, :, :], aT_ps[:])
                    else:
                        nc.scalar.copy(aT_sb[:, ki, :, :], aT_ps[:])
                # mm2: eoT[o, (el, b)] per oi
                for oi in range(2):
                    eo_ps = ps_big.tile([P, 2 * B], F32, tag="big")
                    for el in range(2):
                        for hi in range(2):
                            nc.tensor.matmul(
                                eo_ps[:, ts(el, B)],
                                lhsT=w2_t[
                                    :, hi, el * H + oi * P : el * H + (oi + 1) * P
                                ],
                                rhs=aT_sb[:, hi, el, :],
                                start=(hi == 0),
                                stop=(hi == 1),
                            )
                    dst = eoT_sb[:, oi, e0 : e0 + 2, :]
                    if not use_b2:
                        if oi == 0:
                            nc.vector.tensor_copy(dst, eo_ps[:])
                        else:
                            nc.scalar.copy(dst, eo_ps[:])
                    else:
                        for el in range(2):
                            eng = nc.vector if (oi + el) % 2 == 0 else None
                            if eng is not None:
                                eng.tensor_scalar_add(
                                    eoT_sb[:, oi, e0 + el, :],
                                    eo_ps[:, ts(el, B)],
                                    b2_sb[:, oi, e0 + el : e0 + el + 1],
                                )
                            else:
                                nc.scalar.activation(
                                    out=eoT_sb[:, oi, e0 + el, :],
                                    in_=eo_ps[:, ts(el, B)],
                                    func=AF.Identity,
                                    bias=b2_sb[:, oi, e0 + el : e0 + el + 1],
                                )
                # eo output: f32r bits are valid fp32 -> raw bitcast store
                for el in range(2):
                    nc.gpsimd.dma_start(
                        out=eo_out[e0 + el].rearrange("(oi p) b -> p oi b", p=P),
                        in_=eoT_sb[:, :, e0 + el, :].bitcast(F32),
                    )
                # k~ = Wk @ eoT ; prod = (k~ + bk) * qT ; scores
                prod_sb = work.tile([P, 2, 2, B], DT, tag="prod")
                for oi in range(2):
                    k_ps = ps_big.tile([P, 2 * B], F32, tag="big")
                    for hi in range(2):
                        nc.tensor.matmul(
                            k_ps[:],
                            lhsT=wkt_sb[:, hi, ts(oi, P)],
                            rhs=eoT_sb[:, hi, e0 : e0 + 2, :],
                            start=(hi == 0),
                            stop=(hi == 1),
                        )
                    qtv = qt_sb[:, oi, :]
                    qt_b = bass.AP(
                        tensor=qtv.tensor,
                        offset=qtv.offset,
                        ap=[list(qtv.ap[0]), [0, 2]] + list(qtv.ap[1:]),
                    )
                    nc.vector.scalar_tensor_tensor(
                        out=prod_sb[:, oi, :, :],
                        in0=k_ps[:].rearrange("p (el b) -> p el b", el=2),
                        scalar=bk_sb[:, oi, :],
                        in1=qt_b,
                        op0=ALU.add,
                        op1=ALU.mult,
                    )
                sc_ps = ps_sc.tile([NH, 2 * B], F32, tag="sc")
                for oi in range(2):
                    nc.tensor.matmul(
                        sc_ps[:],
                        lhsT=Lm_sb[:, oi, :],
                        rhs=prod_sb[:, oi, :, :],
                        start=(oi == 0),
                        stop=(oi == 1),
                    )
                nc.scalar.copy(sc_stage[:, e0 : e0 + 2, :], sc_ps[:])

            # ---- load scores dense [q=e*4+n, b], transpose to [b, q] ----
            scd_sb = singles.tile([P, B], F32, tag="scd")
            for n in range(NH):
                nc.gpsimd.dma_start(
                    out=scd_sb[ts(n, ELOC), :], in_=sc_stage[n : n + 1, :, :]
                )
            scn_sb = singles.tile([P, 2, P], F32, tag="scn")
            for half in range(2):
                sc_ps2 = ps_h.tile([P, B], F32, tag="h")
                nc.tensor.transpose(
                    sc_ps2[:, 0:P], scd_sb[:, ts(half, P)], ident_f[:]
                )
                nc.vector.tensor_copy(scn_sb[:, half, :], sc_ps2[:, 0:P])

            cc_ag_in = dram.tile([B, P], F32, tag="ag_in")
            cc_ag_out = dram.tile([NCORES * B, P], F32, tag="ag_out")
            nc.sync.dma_start(
                out=cc_ag_in.rearrange("(h p) q -> p h q", p=P), in_=scn_sb[:]
            )
            nc.gpsimd.collective_compute(
                "AllGather",
                ALU.bypass,
                replica_groups=[list(range(NCORES))],
                ins=[cc_ag_in.opt()],
                outs=[cc_ag_out.opt()],
            )
            gath_sb = singles.tile([P, NCORES, 2, P], F32, tag="gath")
            nc.sync.dma_start(
                out=gath_sb[:],
                in_=cc_ag_out.rearrange("(r h p) q -> p r h q", p=P, h=2),
            )

            # ---- softmax over full expert axis + head mean -> weights ----
            # scores are O(0.02), so the max-subtraction is skipped (exp is
            # safely in range); free index q within a rank block is n*32+e
            wnat_sb = singles.tile([P, 2, B], F32, tag="wnat")
            exp_sb = singles.tile([P, NCORES, 2, P], F32, tag="exp")
            nc.scalar.activation(
                out=exp_sb[:].rearrange("p a b c -> p (a b c)"),
                in_=gath_sb[:].rearrange("p a b c -> p (a b c)"),
                func=AF.Exp,
            )
            for bh in range(2):
                ev = exp_sb[:, :, bh, :].rearrange("p r (n e) -> p r n e", n=NH)
                sm = stats.tile([P, NH], F32, tag="sm")
                for n in range(NH):
                    nc.vector.tensor_reduce(
                        out=sm[:, n : n + 1],
                        in_=ev[:, :, n, :],
                        axis=mybir.AxisListType.XY,
                        op=ALU.add,
                    )
                rsm = stats.tile([P, NH], F32, tag="rsm")
                nc.vector.reciprocal(out=rsm[:], in_=sm[:])
                nc.vector.tensor_scalar_mul(rsm[:], rsm[:], 1.0 / NH)
                nc.vector.memset(wnat_sb[:, bh, :], 0.0)
                wv = wnat_sb[:, bh, :].rearrange("p (r e) -> p r e", r=NCORES)
                for n in range(NH):
                    nc.vector.scalar_tensor_tensor(
                        out=wv,
                        in0=ev[:, :, n, :],
                        scalar=rsm[:, n : n + 1],
                        in1=wv,
                        op0=ALU.mult,
                        op1=ALU.add,
                    )
            nc.sync.dma_start(
                out=w_out.rearrange("(h p) e -> p h e", p=P), in_=wnat_sb[:]
            )

            # ---- wTloc = weights^T restricted to this core's experts ----
            wnat_r = wnat_sb
            if use_f32r:
                wnat_r = singles.tile([P, 2, B], DT, tag="wnat_r")
                for ki in range(2):
                    nc.vector.tensor_copy(wnat_r[:, ki, :], wnat_sb[:, ki, :])
            wT_sb = singles.tile([P, 2, ELOC], F32, tag="wT")
            for mi in range(2):
                wt_ps = ps_h.tile([P, B], F32, tag="h")
                for ki in range(2):
                    nc.tensor.matmul(
                        wt_ps[:, 0:ELOC],
                        lhsT=wnat_r[:, ki, ts(mi, P)],
                        rhs=sel_sb[:, ki, :],
                        start=(ki == 0),
                        stop=(ki == 1),
                    )
                nc.vector.tensor_copy(wT_sb[:, mi, :], wt_ps[:, 0:ELOC])

            # ---- combine: acc[h, b] += eoT_e * wT[:, e] (DVE + GPSIMD split) ----
            acc_v = singles.tile([P, 2, B], F32, tag="acc_v")
            acc_g = singles.tile([P, 2, B], F32, tag="acc_g")
            NDVE = 24
            for oi in range(2):
                nc.vector.memset(acc_v[:, oi, :], 0.0)
                nc.gpsimd.memset(acc_g[:, oi, :], 0.0)
            for e in range(NDVE):
                for oi in range(2):
                    nc.vector.scalar_tensor_tensor(
                        out=acc_v[:, oi, :],
                        in0=eoT_sb[:, oi, e, :],
                        scalar=wT_sb[:, oi, e : e + 1],
                        in1=acc_v[:, oi, :],
                        op0=ALU.mult,
                        op1=ALU.add,
                    )
            gtmp = singles.tile([P, 2, B], F32, tag="gtmp")
            for e in range(NDVE, ELOC):
                for oi in range(2):
                    nc.gpsimd.tensor_scalar_mul(
                        gtmp[:, oi, :], eoT_sb[:, oi, e, :], wT_sb[:, oi, e : e + 1]
                    )
                    nc.gpsimd.tensor_add(
                        acc_g[:, oi, :], acc_g[:, oi, :], gtmp[:, oi, :]
                    )
            acc_r = singles.tile([P, 2, B], DT, tag="acc_r")
            for oi in range(2):
                nc.vector.tensor_add(
                    acc_r[:, oi, :], acc_v[:, oi, :], acc_g[:, oi, :]
                )

            # ---- z_partial = acc @ cw^T (per-core), ReduceScatter over b,
            # ---- then LN+affine+relu on this core's 32-row b-shard ----
            z_sb = singles.tile([P, 2, H], F32, tag="z")
            for mi in range(2):
                z_ps = ps_h.tile([P, 2 * H], F32, tag="h")
                for ki in range(2):
                    nc.tensor.matmul(
                        z_ps[:, 0:H],
                        lhsT=acc_r[:, ki, ts(mi, P)],
                        rhs=cwt_sb[:, ki, :],
                        start=(ki == 0),
                        stop=(ki == 1),
                    )
                nc.vector.tensor_copy(z_sb[:, mi, :], z_ps[:, 0:H])
            cc_rs_in = dram.tile([B, H], F32, tag="rs_in")
            cc_rs_out = dram.tile([B // NCORES, H], F32, tag="rs_out")
            nc.sync.dma_start(
                out=cc_rs_in.rearrange("(mi p) o -> p mi o", p=P), in_=z_sb[:]
            )
            nc.gpsimd.collective_compute(
                "ReduceScatter",
                ALU.add,
                replica_groups=[list(range(NCORES))],
                ins=[cc_rs_in.opt()],
                outs=[cc_rs_out.opt()],
            )
            SH = B // NCORES
            zs_sb = singles.tile([SH, H], F32, tag="zs")
            nc.sync.dma_start(out=zs_sb[:], in_=cc_rs_out[:])
            if use_cb:
                cb_bc32 = singles.tile([SH, H], F32, tag="cb_bc32")
                src = cb_row[:]
                nc.gpsimd.dma_start(
                    out=cb_bc32[:],
                    in_=bass.AP(
                        tensor=src.tensor,
                        offset=src.offset,
                        ap=[[0, SH]] + list(src.ap[1:]),
                    ),
                )
                nc.vector.tensor_add(
                    zs_sb[:], zs_sb[:], cb_bc32[:].bitcast(F32)
                )
            st = stats.tile([P, 6], F32, tag="st")
            nc.vector.bn_stats(out=st[0:SH, :], in_=zs_sb[:])
            mv = stats.tile([P, 2, 2, 2], F32, tag="mv")
            nc.vector.bn_aggr(out=mv[0:SH, 0, 0, :], in_=st[0:SH, :])
            rstd = stats.tile([P, 2, 2], F32, tag="rstd")
            nc.scalar.activation(
                out=rstd[0:SH, 0, 0:1],
                in_=mv[0:SH, 0, 0, 1:2],
                func=AF.Sqrt,
                bias=eps_sb[0:SH, :],
            )
            nc.vector.reciprocal(out=rstd[0:SH, 0, 0:1], in_=rstd[0:SH, 0, 0:1])
            nmr = stats.tile([P, 2, 2], F32, tag="nmr")
            nc.vector.scalar_tensor_tensor(
                out=nmr[0:SH, 0, 0:1],
                in0=mv[0:SH, 0, 0, 0:1],
                scalar=-1.0,
                in1=rstd[0:SH, 0, 0:1],
                op0=ALU.mult,
                op1=ALU.mult,
            )
            y_sb = singles.tile([SH, H], F32, tag="y")
            nc.scalar.activation(
                out=y_sb[:],
                in_=zs_sb[:],
                func=AF.Identity,
                scale=rstd[0:SH, 0, 0:1],
                bias=nmr[0:SH, 0, 0:1],
            )
            nc.vector.tensor_mul(y_sb[:], y_sb[:], cg_bc[0:SH, :])
            nc.vector.tensor_add(y_sb[:], y_sb[:], cbeta_bc[0:SH, :])
            nc.vector.tensor_scalar_max(y_sb[:], y_sb[:], 0.0)
            nc.sync.dma_start(out=y_out[:], in_=y_sb[:])

    nc.compile()
    return nc


_cache = {}


def get_program(flags):
    if flags not in _cache:
        _cache[flags] = build_program(*flags)
    return _cache[flags]


def to_f32r(x):
    """Round fp32 to the PE's f32r format: RNE to 11 explicit mantissa bits
    (bit-exact match to the hardware SWDGE f32->f32r cast)."""
    b = np.ascontiguousarray(x, np.float32).view(np.uint32).astype(np.uint64)
    rb = (b >> 12) & 1
    out = ((b + 0x7FF + rb) & 0xFFFFF000).astype(np.uint32)
    return out.view(np.float32).reshape(x.shape)


def make_in_maps(x, task_embedding, W1, b1, ln1_g, ln1_b, W2, b2,
                 in_proj_w, in_proj_b, cw, cb, cg, cbeta, use_f32r=True):
    f = np.float32
    x = np.asarray(x, f)
    te = np.asarray(task_embedding, f)
    W1 = np.asarray(W1, f)
    b1 = np.asarray(b1, f)
    ln1_g = np.asarray(ln1_g, f)
    ln1_b = np.asarray(ln1_b, f)
    W2 = np.asarray(W2, f)
    b2 = np.asarray(b2, f)
    ipw = np.asarray(in_proj_w, f)
    ipb = np.asarray(in_proj_b, f)
    cw = np.asarray(cw, f)
    cb = np.asarray(cb, f)
    cg = np.asarray(cg, f)
    cbeta = np.asarray(cbeta, f)

    use_b1 = bool(np.any(b1 != 0.0))
    use_b2 = bool(np.any(b2 != 0.0))
    use_affine = bool(np.any(ln1_g != 1.0) or np.any(ln1_b != 0.0))
    use_cb = bool(np.any(cb != 0.0))
    flags = (use_b1, use_b2, use_affine, use_cb, use_f32r)

    Wq, Wk = ipw[:H], ipw[H : 2 * H]
    bq, bk = ipb[:H], ipb[H : 2 * H]

    rr = to_f32r if use_f32r else (lambda a: np.ascontiguousarray(a, f))
    common = {
        "xT": rr(x.T),
        "teT": rr(te.T),
        "wqT": rr(Wq.T),
        "wkT": rr(Wk.T),
        "cwT": rr(cw.T),
        "bqs": np.ascontiguousarray((bq * 0.125).reshape(2, P, 1)),
        "bks": np.ascontiguousarray(bk.reshape(2, P, 1)),
        "ident": np.eye(P, dtype=f),
        "identr": np.eye(P, dtype=f),
        "ones_row": np.ones((1, P), f),
        "cg_row": cg.reshape(1, H).copy(),
        "cbeta_row": cbeta.reshape(1, H).copy(),
    }
    # head mask: Lm[oi][p, n] = 1 if (oi*128+p)//HD == n
    o2 = np.arange(2 * P) // HD
    Lmask = (o2[:, None] == np.arange(NH)[None, :]).astype(f).reshape(2, P, NH)
    common["Lm"] = Lmask
    if use_cb:
        common["cb_row"] = rr(cb.reshape(1, H))

    in_maps = []
    for c in range(NCORES):
        sh = slice(c * ELOC, (c + 1) * ELOC)
        m = dict(common)
        # [pair, i, j, o] = W[2*pair+j][o, i], flattened to [pair, i, 2H]
        m["w1t"] = rr(
            np.ascontiguousarray(
                W1[sh].reshape(ELOC // 2, 2, H, IN).transpose(0, 3, 1, 2)
            ).reshape(ELOC // 2, IN, 2 * H)
        )
        m["w2t"] = rr(
            np.ascontiguousarray(
                W2[sh].reshape(ELOC // 2, 2, H, H).transpose(0, 3, 1, 2)
            ).reshape(ELOC // 2, H, 2 * H)
        )
        m["b2t"] = np.ascontiguousarray(b2[sh].T.reshape(2, P, ELOC))
        selm = np.zeros((B, ELOC), f)
        selm[np.arange(c * ELOC, (c + 1) * ELOC), np.arange(ELOC)] = 1.0
        m["sel"] = selm
        if use_b1:
            m["b1r"] = rr(b1[sh].reshape(ELOC // 2, 1, 2 * H))
        if use_affine:
            m["lng"] = np.ascontiguousarray(ln1_g[sh])
            m["lnb"] = np.ascontiguousarray(ln1_b[sh])
        in_maps.append(m)
    return flags, in_maps


def assemble(results):
    eo = np.concatenate(
        [res["eo_out"].transpose(2, 0, 1) for res in results], axis=1
    )
    weights = results[0]["w_out"][:, None, :]
    out = np.concatenate([res["y_out"] for res in results], axis=0)
    return out, weights, eo


def run(inputs, trace=False, use_f32r=True, **kw):
    flags, in_maps = make_in_maps(**inputs, use_f32r=use_f32r)
    nc = get_program(flags)
    res = run_bass_kernel_spmd(
        nc, in_maps, core_ids=list(range(NCORES)), trace=trace, **kw
    )
    return assemble(res.results), res


def kernel(**inputs):
    outs, _ = run(inputs)
    return outs
